# revision 16
# baseline (speedup 1.0000x reference)
"""Trainium2 Bass kernel for nn_Base2DInference (sampling).

Data-parallel over the sample batch B across 8 NeuronCores. Per core:
  - tiny MLP 10->32->32->32->32 in fp32 on the PE, 4 sample-groups packed
    into the 128x128 array via tile_position (K,M=32 blocks)
  - PE transpose of the MLP head to samples-on-partitions layout
  - index math (rotation, texel indices, z bin) on DVE/ACT
  - fac/pdf-norm table lookups as 8-way arithmetic selects
  - per-texel gather from the replicated DRAM pdf via indirect DMA
  - weighted mixture reduction -> [B/8] output
"""
import sys, os, time, types

sys.path.insert(0, '/opt/trn_rl_repo')

import numpy as np


def _install_ntff_hook_shim():
    if 'antenv.axon_hooks' in sys.modules:
        return
    try:
        from trn_agent_boot.trn_boot import _ntff_profile_via_ctypes
        hook = _ntff_profile_via_ctypes('/opt/axon/libaxon_pjrt.so')
    except Exception:
        hook = None
    mod = types.ModuleType('antenv.axon_hooks')
    _state = {'hook': hook}
    mod.set_axon_ntff_profile_hook = lambda h: _state.__setitem__('hook', h)
    mod.get_axon_ntff_profile_hook = lambda: _state['hook']
    sys.modules['antenv.axon_hooks'] = mod


_install_ntff_hook_shim()

from concourse import bass, mybir, bacc, tile
from concourse.bass_utils import run_bass_kernel_spmd

F32 = mybir.dt.float32
I32 = mybir.dt.int32

RES, ANG, D, HID, CIN = 512, 8, 8, 32, 10
B = 1048576
NC_N = 8
BC = B // NC_N            # samples per core: 131072
NQ = 4                    # packed sample groups per matmul
QS = BC // NQ             # samples per group: 32768
NT = QS // 512            # super-tiles: 64
NCHUNK = BC // 128        # 128-sample chunks: 1024

_MAXW = 1


def _patched_drain_and_barrier(self, tick_clock, wait_clock):
    from concourse.vector_clock import ScopedClock
    nc = self.nc
    drain_inst = nc.sync.drain()
    wait_clock.add_sem_waits(drain_inst.ins, ScopedClock({None: tick_clock.global_clock}))
    si = drain_inst.ins.sync_info
    if si is not None and si.on_wait and len(si.on_wait) > _MAXW:
        waits = list(si.on_wait)
        drain_inst.ins.sync_info = mybir.SyncInfo(
            on_wait=waits[:_MAXW], on_update=list(si.on_update))
        for i in range(_MAXW, len(waits), _MAXW):
            nop = nc.sync.nop(nofuse=True)
            nop.ins.sync_info = mybir.SyncInfo(on_wait=waits[i:i + _MAXW], on_update=[])
    nc.all_engine_barrier()
    popped = nc._tile_sem_poison_stack.pop()
    assert popped is self._sem_poison
    nc.clear_and_free_semaphores(list(self.sems.allocated().values()))
    nc.all_engine_barrier()


tile.TileContext._drain_and_barrier = _patched_drain_and_barrier


def split_sync_waits(nc, max_waits=_MAXW):
    fn = nc.m.functions[0]
    root_bb = nc.cur_bb.bb
    for bb in list(fn.blocks):
        insts = bb.instructions
        changed = False
        out = []
        for inst in insts:
            si = inst.sync_info
            waits = list(si.on_wait) if (si is not None and si.on_wait) else []
            if len(waits) > max_waits:
                changed = True
                extra = waits[:-max_waits]
                inst.sync_info = mybir.SyncInfo(
                    on_wait=waits[-max_waits:], on_update=list(si.on_update))
                for j in range(0, len(extra), max_waits):
                    nop = nc.engines[inst.engine].nop(nofuse=True)
                    nop.ins.sync_info = mybir.SyncInfo(
                        on_wait=extra[j:j + max_waits], on_update=[])
                    root_insts = root_bb.instructions
                    assert root_insts[-1].name == nop.ins.name
                    root_bb.instructions = root_insts[:-1]
                    out.append(nop.ins)
            out.append(inst)
        if changed:
            bb.instructions = out


class PatchedBacc(bacc.Bacc):
    def finalize(self):
        self.compile()
        split_sync_waits(self)
        self.verify_switch_hints()
        self.assert_all_executable()
        self.freeze()
        self._finalized = True


def build_kernel():
    AL = mybir.AluOpType
    AF = mybir.ActivationFunctionType
    nc = PatchedBacc()

    cond_t = nc.declare_dram_parameter("cond_t", [NQ * CIN, QS], F32, isOutput=False)
    wi_x = nc.declare_dram_parameter("wi_x", [128, NCHUNK * D], F32, isOutput=False)
    wi_y = nc.declare_dram_parameter("wi_y", [128, NCHUNK * D], F32, isOutput=False)
    w0p = nc.declare_dram_parameter("w0p", [128, 128], F32, isOutput=False)
    w1p = nc.declare_dram_parameter("w1p", [128, 128], F32, isOutput=False)
    w2p = nc.declare_dram_parameter("w2p", [128, 128], F32, isOutput=False)
    w3p = nc.declare_dram_parameter("w3p", [128, 128], F32, isOutput=False)
    b012 = nc.declare_dram_parameter("b012", [128, 3], F32, isOutput=False)
    b3p = nc.declare_dram_parameter("b3p", [128, 1], F32, isOutput=False)
    facb = nc.declare_dram_parameter("facb", [128, ANG * D], F32, isOutput=False)
    dcol = nc.declare_dram_parameter("dcol", [128, D], F32, isOutput=False)
    ident = nc.declare_dram_parameter("ident", [128, 128], F32, isOutput=False)
    pdf_flat = nc.declare_dram_parameter("pdf_flat", [ANG * D * RES * RES, 1], F32,
                                         isOutput=False)
    pdf2d = nc.declare_dram_parameter("pdf2d", [ANG * D * 128, RES * RES // 128], F32,
                                      isOutput=False)
    out_ext = nc.declare_dram_parameter("out", [128, NCHUNK], F32, isOutput=True)

    with tile.TileContext(nc) as tc:
        with (
            tc.tile_pool(name="const", bufs=1) as cpool,
            tc.tile_pool(name="work", bufs=2) as wpool,
            tc.tile_pool(name="psum", bufs=2, space="PSUM") as ppool,
            tc.tile_pool(name="psum2", bufs=2, space="PSUM") as ppool2,
        ):
            # ---- constants to SBUF ----
            w0t = cpool.tile([128, 128], F32); nc.sync.dma_start(w0t[:], w0p[:])
            w1t = cpool.tile([128, 128], F32); nc.sync.dma_start(w1t[:], w1p[:])
            w2t = cpool.tile([128, 128], F32); nc.sync.dma_start(w2t[:], w2p[:])
            w3t = cpool.tile([128, 128], F32); nc.sync.dma_start(w3t[:], w3p[:])
            bt = cpool.tile([128, 3], F32); nc.sync.dma_start(bt[:], b012[:])
            b3t = cpool.tile([128, 1], F32); nc.sync.dma_start(b3t[:], b3p[:])
            fact = cpool.tile([128, ANG * D], F32); nc.sync.dma_start(fact[:], facb[:])
            dct = cpool.tile([128, D], F32); nc.sync.dma_start(dct[:], dcol[:])
            idt = cpool.tile([128, 128], F32); nc.sync.dma_start(idt[:], ident[:])
            rhs = cpool.tile([128, 512], F32)
            nc.vector.memset(rhs[:], 0.0)

            # ---- per-texture pdf sums on PE -> norm table [128, 64] ----
            # pdf2d rows: texture t occupies rows 128t..128t+127, cols 2048
            ones = cpool.tile([128, 1], F32)
            nc.vector.memset(ones[:], 1.0)
            srow = cpool.tile([1, ANG * D], F32)
            for t in range(ANG * D):
                pt = wpool.tile([128, 2048], F32, tag="pdfsum")
                nc.sync.dma_start(pt[:], pdf2d[t * 128:(t + 1) * 128, :])
                ps = ppool2.tile([1, 512], F32, space="PSUM", tag="pssum")
                for c in range(4):
                    nc.tensor.matmul(ps[:], ones[:], pt[:, c * 512:(c + 1) * 512],
                                     start=(c == 0), stop=(c == 3))
                nc.vector.tensor_reduce(srow[:, t:t + 1], ps[:],
                                        axis=mybir.AxisListType.X, op=AL.add)
            # norm = 65536 / max(sum, 1e-12)
            nsr = cpool.tile([1, ANG * D], F32)
            nc.vector.tensor_scalar_max(nsr[:], srow[:], 1e-12)
            nc.vector.reciprocal(nsr[:], nsr[:])
            nc.vector.tensor_scalar_mul(nsr[:], nsr[:], float(RES * RES) / 4.0)
            normt = cpool.tile([128, ANG * D], F32)
            nc.gpsimd.partition_broadcast(normt[:], nsr[:], channels=128)

            ot = cpool.tile([128, NCHUNK], F32)
            fi_all = cpool.tile([128, NCHUNK * D], I32)
            pv_all = cpool.tile([128, NCHUNK * D], F32)
            wn_all = cpool.tile([128, NCHUNK * D], F32)
            den_all = cpool.tile([128, NCHUNK], F32)

            for s in range(NT):
                # ---- MLP: block-diagonal weights, one matmul per layer ----
                for g in range(NQ):
                    nc.sync.dma_start(rhs[32 * g:32 * g + CIN, :],
                                      cond_t[g * CIN:(g + 1) * CIN,
                                             s * 512:(s + 1) * 512])
                h = rhs
                for li, wt_ in enumerate((w0t, w1t, w2t, w3t)):
                    mm = ppool.tile([128, 512], F32, space="PSUM", tag="mm")
                    nc.tensor.matmul(mm[:], wt_[:], h[:], start=True, stop=True)
                    hn = wpool.tile([128, 512], F32, tag=f"h{li % 2}")
                    if li < 3:
                        nc.scalar.activation(hn[:], mm[:], AF.Relu,
                                             bias=bt[:, li:li + 1], scale=1.0)
                    else:
                        nc.vector.tensor_scalar_add(hn[:], mm[:], b3t[:, 0:1])
                    h = hn

                # ---- transpose 4x [128,128] -> chunks ch = c*4+g ----
                tp = ppool.tile([128, 512], F32, space="PSUM", tag="tp")
                for c in range(4):
                    nc.tensor.transpose(
                        tp[:, c * 128:(c + 1) * 128],
                        h[:, c * 128:(c + 1) * 128],
                        idt[:])
                tps = wpool.tile([128, 512], F32, tag="tps")
                nc.vector.tensor_copy(tps[:], tp[:])

                def blk(base):
                    return tps[:].rearrange("p (ch f) -> p ch f", ch=16)[:, :, base:base + 8]

                WT, VX, VY, ZZ = blk(0), blk(8), blk(16), blk(24)
                cw = s * 16 * D
                wxs = wpool.tile([128, 128], F32, tag="wxs")
                wys = wpool.tile([128, 128], F32, tag="wys")
                nc.sync.dma_start(wxs[:], wi_x[:, cw:cw + 128])
                nc.sync.dma_start(wys[:], wi_y[:, cw:cw + 128])
                wxs3 = wxs[:].rearrange("p (ch f) -> p ch f", ch=16)
                wys3 = wys[:].rearrange("p (ch f) -> p ch f", ch=16)

                t1 = wpool.tile([128, 16, 8], F32, tag="t1")
                t2 = wpool.tile([128, 16, 8], F32, tag="t2")
                n2 = wpool.tile([128, 16, 8], F32, tag="n2")
                nc.scalar.activation(t1[:], VX, AF.Square)
                nc.scalar.activation(t2[:], VY, AF.Square)
                nc.vector.tensor_tensor(n2[:], t1[:], t2[:], op=AL.add)
                nc.scalar.activation(n2[:], n2[:], AF.Sqrt)
                nc.vector.tensor_scalar_max(n2[:], n2[:], 1e-12)
                inv = wpool.tile([128, 16, 8], F32, tag="inv")
                nc.vector.reciprocal(inv[:], n2[:])

                rx = wpool.tile([128, 16, 8], F32, tag="rx")
                ry = wpool.tile([128, 16, 8], F32, tag="ry")
                nc.vector.tensor_tensor(t1[:], VX, wxs3, op=AL.mult)
                nc.vector.tensor_tensor(t2[:], VY, wys3, op=AL.mult)
                nc.vector.tensor_tensor(rx[:], t1[:], t2[:], op=AL.subtract)
                nc.vector.tensor_tensor(t1[:], VY, wxs3, op=AL.mult)
                nc.vector.tensor_tensor(t2[:], VX, wys3, op=AL.mult)
                nc.vector.tensor_tensor(ry[:], t1[:], t2[:], op=AL.add)
                nc.vector.tensor_tensor(rx[:], rx[:], inv[:], op=AL.mult)
                nc.vector.tensor_tensor(ry[:], ry[:], inv[:], op=AL.mult)
                nc.vector.tensor_scalar(rx[:], rx[:], 256.0, 256.0,
                                        op0=AL.mult, op1=AL.add)
                nc.vector.tensor_scalar(rx[:], rx[:], 0.0, 511.0,
                                        op0=AL.max, op1=AL.min)
                nc.vector.tensor_scalar(ry[:], ry[:], 256.0, 256.0,
                                        op0=AL.mult, op1=AL.add)
                nc.vector.tensor_scalar(ry[:], ry[:], 0.0, 511.0,
                                        op0=AL.max, op1=AL.min)
                ixi = wpool.tile([128, 16, 8], I32, tag="ixi")
                iyi = wpool.tile([128, 16, 8], I32, tag="iyi")
                flr = wpool.tile([128, 16, 8], F32, tag="flr")
                fmk = wpool.tile([128, 16, 8], F32, tag="fmk")

                def exact_floor(dst_i, src_f):
                    # dst = floor(src) for src >= 0, independent of cvt rounding
                    nc.vector.tensor_copy(dst_i[:], src_f)
                    nc.vector.tensor_copy(flr[:], dst_i[:])
                    nc.vector.tensor_tensor(fmk[:], flr[:], src_f, op=AL.is_gt)
                    nc.vector.tensor_tensor(flr[:], flr[:], fmk[:], op=AL.subtract)
                    nc.vector.tensor_copy(dst_i[:], flr[:])

                exact_floor(ixi, rx[:])
                exact_floor(iyi, ry[:])

                sig = wpool.tile([128, 16, 8], F32, tag="sig")
                nc.scalar.activation(sig[:], ZZ, AF.Sigmoid)
                nc.vector.tensor_scalar(sig[:], sig[:], float(ANG), 0.5,
                                        op0=AL.mult, op1=AL.add)
                nc.vector.tensor_scalar_min(sig[:], sig[:], 7.9)
                zif = wpool.tile([128, 16, 8], F32, tag="zif")
                zii = wpool.tile([128, 16, 8], I32, tag="zii")
                exact_floor(zii, sig[:])
                nc.vector.tensor_copy(zif[:], zii[:])

                fi3 = fi_all[:, cw:cw + 128].rearrange("p (ch f) -> p ch f", ch=16)
                dcs = dct[:, 0:8].rearrange("p (o f) -> p o f", o=1).to_broadcast([128, 16, 8])
                ixf = wpool.tile([128, 16, 8], F32, tag="ixf")
                iyf = wpool.tile([128, 16, 8], F32, tag="iyf")
                nc.vector.tensor_copy(ixf[:], ixi[:])
                nc.vector.tensor_copy(iyf[:], iyi[:])
                ff = wpool.tile([128, 16, 8], F32, tag="ff")
                nc.vector.scalar_tensor_tensor(ff[:], zif[:], 8.0, dcs,
                                               op0=AL.mult, op1=AL.add)
                nc.vector.scalar_tensor_tensor(ff[:], ff[:], 512.0, iyf[:],
                                               op0=AL.mult, op1=AL.add)
                nc.vector.scalar_tensor_tensor(ff[:], ff[:], 512.0, ixf[:],
                                               op0=AL.mult, op1=AL.add)
                nc.vector.tensor_copy(fi3, ff[:])

                facv = wpool.tile([128, 16, 8], F32, tag="facv")
                nrmv = wpool.tile([128, 16, 8], F32, tag="nrmv")
                msk = wpool.tile([128, 16, 8], F32, tag="msk")
                tmpm = wpool.tile([128, 16, 8], F32, tag="tmpm")
                nc.vector.memset(facv[:], 0.0)
                nc.vector.memset(nrmv[:], 0.0)
                for k in range(ANG):
                    fr = fact[:, k * D:(k + 1) * D].rearrange(
                        "p (o f) -> p o f", o=1).to_broadcast([128, 16, 8])
                    nr = normt[:, k * D:(k + 1) * D].rearrange(
                        "p (o f) -> p o f", o=1).to_broadcast([128, 16, 8])
                    nc.vector.tensor_scalar(msk[:], zif[:], float(k), None,
                                            op0=AL.is_equal)
                    nc.vector.tensor_tensor(tmpm[:], msk[:], fr, op=AL.mult)
                    nc.vector.tensor_tensor(facv[:], facv[:], tmpm[:], op=AL.add)
                    nc.vector.tensor_tensor(tmpm[:], msk[:], nr, op=AL.mult)
                    nc.vector.tensor_tensor(nrmv[:], nrmv[:], tmpm[:], op=AL.add)

                rl = wpool.tile([128, 16, 8], F32, tag="rl")
                wd = wpool.tile([128, 16, 8], F32, tag="wd")
                nc.scalar.activation(rl[:], WT, AF.Relu)
                wn3 = wn_all[:, cw:cw + 128].rearrange("p (ch f) -> p ch f", ch=16)
                nc.vector.tensor_tensor(wn3, rl[:], facv[:], op=AL.mult)
                nc.scalar.activation(wd[:], facv[:], AF.Abs)
                nc.vector.tensor_tensor(wd[:], rl[:], wd[:], op=AL.mult)
                den3 = den_all[:, s * 16:(s + 1) * 16].rearrange("p (c o) -> p c o", o=1)
                nc.vector.tensor_reduce(den3, wd[:],
                                        axis=mybir.AxisListType.X, op=AL.add)
                # fold the norm factor into wn
                nc.vector.tensor_tensor(wn3, wn3, nrmv[:], op=AL.mult)

            # ---- phase 2: gather loop (static indirect APs on staging tiles) ----
            UNROLL = 64
            fi_st = cpool.tile([128, UNROLL], I32)
            pv_st = cpool.tile([128, UNROLL], F32)
            with tc.For_i(0, NCHUNK * D // UNROLL, 1) as iv:
                nc.vector.tensor_copy(fi_st[:], fi_all[:, bass.ts(iv, UNROLL)])
                for k in range(UNROLL):
                    nc.gpsimd.indirect_dma_start(
                        out=pv_st[:, k:k + 1], out_offset=None,
                        in_=pdf_flat[:],
                        in_offset=bass.IndirectOffsetOnAxis(
                            ap=fi_st[:, k:k + 1], axis=0))
                nc.vector.tensor_copy(pv_all[:, bass.ts(iv, UNROLL)], pv_st[:])

            # ---- phase 3: combine ----
            num = cpool.tile([128, NCHUNK * D], F32)
            nc.vector.tensor_tensor(num[:], wn_all[:], pv_all[:], op=AL.mult)
            nc.vector.tensor_reduce(
                ot[:].rearrange("p (c o) -> p c o", o=1),
                num[:].rearrange("p (c f) -> p c f", f=D),
                axis=mybir.AxisListType.X, op=AL.add)
            nc.vector.tensor_scalar_max(den_all[:], den_all[:], 1e-12)
            nc.vector.reciprocal(den_all[:], den_all[:])
            nc.vector.tensor_tensor(ot[:], ot[:], den_all[:], op=AL.mult)
            nc.sync.dma_start(out_ext[:], ot[:])
    return nc


def prep_inputs(wi, cond, w0, b0, w1, b1, w2, b2, w3, b3, pdf, fac):
    """Host-side sharding + layout. Returns in_maps (list of 8 dicts)."""
    perm = np.concatenate([np.arange(D),                 # weight
                           D + 2 * np.arange(D),         # vx
                           D + 2 * np.arange(D) + 1,     # vy
                           3 * D + np.arange(D)])        # z
    w3r = w3[:, perm].astype(np.float32)
    b3r = b3[perm].astype(np.float32)

    def packw(w, kk):
        t = np.zeros((128, 128), np.float32)
        for g in range(NQ):
            t[32 * g:32 * g + kk, 32 * g:32 * g + 32] = w
        return t

    w0p = packw(w0, CIN); w1p = packw(w1, HID)
    w2p = packw(w2, HID); w3p = packw(w3r, HID)
    b012 = np.zeros((128, 3), np.float32)
    b3p = np.zeros((128, 1), np.float32)
    for g in range(NQ):
        for li, b in enumerate((b0, b1, b2)):
            b012[32 * g:32 * g + 32, li] = b
        b3p[32 * g:32 * g + 32, 0] = b3r
    facb = np.broadcast_to(fac.reshape(1, ANG * D), (128, ANG * D)).copy()
    dcol = np.broadcast_to(np.arange(D, dtype=np.float32).reshape(1, D),
                           (128, D)).copy()
    ident = np.eye(128, dtype=np.float32)
    pdf_flat = pdf.reshape(-1, 1).astype(np.float32)
    pdf2d = pdf.reshape(ANG * D * 128, RES * RES // 128).astype(np.float32)

    in_maps = []
    for c in range(NC_N):
        sl = slice(c * BC, (c + 1) * BC)
        cond_c = cond[sl].reshape(NQ, QS, CIN)
        cond_t = np.ascontiguousarray(
            cond_c.transpose(0, 2, 1).reshape(NQ * CIN, QS))
        # wi per chunk layout: chunk (s,g,cc): sample g*QS + s*512 + cc*128 + p
        wi_c = wi[sl]
        g_, s_, cc_, p_ = np.meshgrid(np.arange(NQ), np.arange(NT), np.arange(4),
                                      np.arange(128), indexing='ij')
        samp = (g_ * QS + s_ * 512 + cc_ * 128 + p_)
        chunk = (s_ * 16 + cc_ * 4 + g_)
        wx = np.zeros((128, NCHUNK, D), np.float32)
        wy = np.zeros((128, NCHUNK, D), np.float32)
        wx[p_.ravel(), chunk.ravel()] = wi_c[samp.ravel(), 0:1]
        wy[p_.ravel(), chunk.ravel()] = wi_c[samp.ravel(), 1:2]
        in_maps.append(dict(
            cond_t=cond_t, wi_x=wx.reshape(128, NCHUNK * D),
            wi_y=wy.reshape(128, NCHUNK * D),
            w0p=w0p, w1p=w1p, w2p=w2p, w3p=w3p, b012=b012, b3p=b3p,
            facb=facb, dcol=dcol, ident=ident,
            pdf_flat=pdf_flat, pdf2d=pdf2d))
    return in_maps


def unshard_output(results):
    out = np.empty(B, np.float32)
    g_, s_, cc_, p_ = np.meshgrid(np.arange(NQ), np.arange(NT), np.arange(4),
                                  np.arange(128), indexing='ij')
    samp = (g_ * QS + s_ * 512 + cc_ * 128 + p_).ravel()
    chunk = (s_ * 16 + cc_ * 4 + g_).ravel()
    for c in range(NC_N):
        o = results[c]["out"]  # [128, NCHUNK]
        out[c * BC + samp] = o[p_.ravel(), chunk]
    return out


_CACHE = {}


def kernel(**inputs):
    if 'nc' not in _CACHE:
        _CACHE['nc'] = build_kernel()
    nc = _CACHE['nc']
    if not nc.is_finalized():
        nc.finalize()
    in_maps = prep_inputs(**{k: np.asarray(v) for k, v in inputs.items()})
    r = run_bass_kernel_spmd(nc, in_maps, list(range(NC_N)),
                             trace=bool(os.environ.get("KTRACE")))
    if r.exec_time_ns:
        print(f"HW exec time: {r.exec_time_ns} ns")
    return unshard_output(r.results)


if __name__ == "__main__":
    pass


# revision 17
# speedup vs baseline: 1.0008x; 1.0008x over previous
"""Trainium2 Bass kernel for nn_Base2DInference (sampling).

Data-parallel over the sample batch B across 8 NeuronCores. Per core:
  - tiny MLP 10->32->32->32->32 in fp32 on the PE, 4 sample-groups packed
    into the 128x128 array via tile_position (K,M=32 blocks)
  - PE transpose of the MLP head to samples-on-partitions layout
  - index math (rotation, texel indices, z bin) on DVE/ACT
  - fac/pdf-norm table lookups as 8-way arithmetic selects
  - per-texel gather from the replicated DRAM pdf via indirect DMA
  - weighted mixture reduction -> [B/8] output
"""
import sys, os, time, types

sys.path.insert(0, '/opt/trn_rl_repo')

import numpy as np


def _install_ntff_hook_shim():
    if 'antenv.axon_hooks' in sys.modules:
        return
    try:
        from trn_agent_boot.trn_boot import _ntff_profile_via_ctypes
        hook = _ntff_profile_via_ctypes('/opt/axon/libaxon_pjrt.so')
    except Exception:
        hook = None
    mod = types.ModuleType('antenv.axon_hooks')
    _state = {'hook': hook}
    mod.set_axon_ntff_profile_hook = lambda h: _state.__setitem__('hook', h)
    mod.get_axon_ntff_profile_hook = lambda: _state['hook']
    sys.modules['antenv.axon_hooks'] = mod


_install_ntff_hook_shim()

from concourse import bass, mybir, bacc, tile
from concourse.bass_utils import run_bass_kernel_spmd

F32 = mybir.dt.float32
I32 = mybir.dt.int32

RES, ANG, D, HID, CIN = 512, 8, 8, 32, 10
B = 1048576
NC_N = 8
BC = B // NC_N            # samples per core: 131072
NQ = 4                    # packed sample groups per matmul
QS = BC // NQ             # samples per group: 32768
NT = QS // 512            # super-tiles: 64
NCHUNK = BC // 128        # 128-sample chunks: 1024

_MAXW = 1


def _patched_drain_and_barrier(self, tick_clock, wait_clock):
    from concourse.vector_clock import ScopedClock
    nc = self.nc
    drain_inst = nc.sync.drain()
    wait_clock.add_sem_waits(drain_inst.ins, ScopedClock({None: tick_clock.global_clock}))
    si = drain_inst.ins.sync_info
    if si is not None and si.on_wait and len(si.on_wait) > _MAXW:
        waits = list(si.on_wait)
        drain_inst.ins.sync_info = mybir.SyncInfo(
            on_wait=waits[:_MAXW], on_update=list(si.on_update))
        for i in range(_MAXW, len(waits), _MAXW):
            nop = nc.sync.nop(nofuse=True)
            nop.ins.sync_info = mybir.SyncInfo(on_wait=waits[i:i + _MAXW], on_update=[])
    nc.all_engine_barrier()
    popped = nc._tile_sem_poison_stack.pop()
    assert popped is self._sem_poison
    nc.clear_and_free_semaphores(list(self.sems.allocated().values()))
    nc.all_engine_barrier()


tile.TileContext._drain_and_barrier = _patched_drain_and_barrier


def split_sync_waits(nc, max_waits=_MAXW):
    fn = nc.m.functions[0]
    root_bb = nc.cur_bb.bb
    for bb in list(fn.blocks):
        insts = bb.instructions
        changed = False
        out = []
        for inst in insts:
            si = inst.sync_info
            waits = list(si.on_wait) if (si is not None and si.on_wait) else []
            if len(waits) > max_waits:
                changed = True
                extra = waits[:-max_waits]
                inst.sync_info = mybir.SyncInfo(
                    on_wait=waits[-max_waits:], on_update=list(si.on_update))
                for j in range(0, len(extra), max_waits):
                    nop = nc.engines[inst.engine].nop(nofuse=True)
                    nop.ins.sync_info = mybir.SyncInfo(
                        on_wait=extra[j:j + max_waits], on_update=[])
                    root_insts = root_bb.instructions
                    assert root_insts[-1].name == nop.ins.name
                    root_bb.instructions = root_insts[:-1]
                    out.append(nop.ins)
            out.append(inst)
        if changed:
            bb.instructions = out


class PatchedBacc(bacc.Bacc):
    def finalize(self):
        self.compile()
        split_sync_waits(self)
        self.verify_switch_hints()
        self.assert_all_executable()
        self.freeze()
        self._finalized = True


def build_kernel():
    AL = mybir.AluOpType
    AF = mybir.ActivationFunctionType
    nc = PatchedBacc()

    cond_t = nc.declare_dram_parameter("cond_t", [NQ * CIN, QS], F32, isOutput=False)
    wi_x = nc.declare_dram_parameter("wi_x", [128, NCHUNK * D], F32, isOutput=False)
    wi_y = nc.declare_dram_parameter("wi_y", [128, NCHUNK * D], F32, isOutput=False)
    w0p = nc.declare_dram_parameter("w0p", [128, 128], F32, isOutput=False)
    w1p = nc.declare_dram_parameter("w1p", [128, 128], F32, isOutput=False)
    w2p = nc.declare_dram_parameter("w2p", [128, 128], F32, isOutput=False)
    w3p = nc.declare_dram_parameter("w3p", [128, 128], F32, isOutput=False)
    b012 = nc.declare_dram_parameter("b012", [128, 3], F32, isOutput=False)
    b3p = nc.declare_dram_parameter("b3p", [128, 1], F32, isOutput=False)
    facb = nc.declare_dram_parameter("facb", [128, ANG * D], F32, isOutput=False)
    dcol = nc.declare_dram_parameter("dcol", [128, D], F32, isOutput=False)
    ident = nc.declare_dram_parameter("ident", [128, 128], F32, isOutput=False)
    pdf_flat = nc.declare_dram_parameter("pdf_flat", [ANG * D * RES * RES, 1], F32,
                                         isOutput=False)
    pdf2d = nc.declare_dram_parameter("pdf2d", [ANG * D * 128, RES * RES // 128], F32,
                                      isOutput=False)
    out_ext = nc.declare_dram_parameter("out", [128, NCHUNK], F32, isOutput=True)

    with tile.TileContext(nc) as tc:
        with (
            tc.tile_pool(name="const", bufs=1) as cpool,
            tc.tile_pool(name="work", bufs=2) as wpool,
            tc.tile_pool(name="psum", bufs=2, space="PSUM") as ppool,
            tc.tile_pool(name="psum2", bufs=2, space="PSUM") as ppool2,
        ):
            # ---- constants to SBUF ----
            w0t = cpool.tile([128, 128], F32); nc.sync.dma_start(w0t[:], w0p[:])
            w1t = cpool.tile([128, 128], F32); nc.sync.dma_start(w1t[:], w1p[:])
            w2t = cpool.tile([128, 128], F32); nc.sync.dma_start(w2t[:], w2p[:])
            w3t = cpool.tile([128, 128], F32); nc.sync.dma_start(w3t[:], w3p[:])
            bt = cpool.tile([128, 3], F32); nc.sync.dma_start(bt[:], b012[:])
            b3t = cpool.tile([128, 1], F32); nc.sync.dma_start(b3t[:], b3p[:])
            fact = cpool.tile([128, ANG * D], F32); nc.sync.dma_start(fact[:], facb[:])
            dct = cpool.tile([128, D], F32); nc.sync.dma_start(dct[:], dcol[:])
            idt = cpool.tile([128, 128], F32); nc.sync.dma_start(idt[:], ident[:])
            rhs = cpool.tile([128, 512], F32)
            nc.vector.memset(rhs[:], 0.0)

            # ---- per-texture pdf sums on PE -> norm table [128, 64] ----
            # pdf2d rows: texture t occupies rows 128t..128t+127, cols 2048
            ones = cpool.tile([128, 1], F32)
            nc.vector.memset(ones[:], 1.0)
            srow = cpool.tile([1, ANG * D], F32)
            for t in range(ANG * D):
                pt = wpool.tile([128, 2048], F32, tag="pdfsum")
                nc.sync.dma_start(pt[:], pdf2d[t * 128:(t + 1) * 128, :])
                ps = ppool2.tile([1, 512], F32, space="PSUM", tag="pssum")
                for c in range(4):
                    nc.tensor.matmul(ps[:], ones[:], pt[:, c * 512:(c + 1) * 512],
                                     start=(c == 0), stop=(c == 3))
                nc.vector.tensor_reduce(srow[:, t:t + 1], ps[:],
                                        axis=mybir.AxisListType.X, op=AL.add)
            # norm = 65536 / max(sum, 1e-12)
            nsr = cpool.tile([1, ANG * D], F32)
            nc.vector.tensor_scalar_max(nsr[:], srow[:], 1e-12)
            nc.vector.reciprocal(nsr[:], nsr[:])
            nc.vector.tensor_scalar_mul(nsr[:], nsr[:], float(RES * RES) / 4.0)
            normt = cpool.tile([128, ANG * D], F32)
            nc.gpsimd.partition_broadcast(normt[:], nsr[:], channels=128)

            ot = cpool.tile([128, NCHUNK], F32)
            fi_all = cpool.tile([128, NCHUNK * D], I32)
            pv_all = cpool.tile([128, NCHUNK * D], F32)
            wn_all = cpool.tile([128, NCHUNK * D], F32)
            den_all = cpool.tile([128, NCHUNK], F32)

            for s in range(NT):
                # ---- MLP: block-diagonal weights, one matmul per layer ----
                for g in range(NQ):
                    nc.sync.dma_start(rhs[32 * g:32 * g + CIN, :],
                                      cond_t[g * CIN:(g + 1) * CIN,
                                             s * 512:(s + 1) * 512])
                h = rhs
                for li, wt_ in enumerate((w0t, w1t, w2t, w3t)):
                    mm = ppool.tile([128, 512], F32, space="PSUM", tag="mm")
                    nc.tensor.matmul(mm[:], wt_[:], h[:], start=True, stop=True)
                    hn = wpool.tile([128, 512], F32, tag=f"h{li % 2}")
                    if li < 3:
                        nc.scalar.activation(hn[:], mm[:], AF.Relu,
                                             bias=bt[:, li:li + 1], scale=1.0)
                    else:
                        nc.vector.tensor_scalar_add(hn[:], mm[:], b3t[:, 0:1])
                    h = hn

                # ---- transpose 4x [128,128] -> chunks ch = c*4+g ----
                tp = ppool.tile([128, 512], F32, space="PSUM", tag="tp")
                for c in range(4):
                    nc.tensor.transpose(
                        tp[:, c * 128:(c + 1) * 128],
                        h[:, c * 128:(c + 1) * 128],
                        idt[:])
                tps = wpool.tile([128, 512], F32, tag="tps")
                nc.vector.tensor_copy(tps[:], tp[:])

                def blk(base):
                    return tps[:].rearrange("p (ch f) -> p ch f", ch=16)[:, :, base:base + 8]

                WT, VX, VY, ZZ = blk(0), blk(8), blk(16), blk(24)
                cw = s * 16 * D
                wxs = wpool.tile([128, 128], F32, tag="wxs")
                wys = wpool.tile([128, 128], F32, tag="wys")
                nc.sync.dma_start(wxs[:], wi_x[:, cw:cw + 128])
                nc.sync.dma_start(wys[:], wi_y[:, cw:cw + 128])
                wxs3 = wxs[:].rearrange("p (ch f) -> p ch f", ch=16)
                wys3 = wys[:].rearrange("p (ch f) -> p ch f", ch=16)

                t1 = wpool.tile([128, 16, 8], F32, tag="t1")
                t2 = wpool.tile([128, 16, 8], F32, tag="t2")
                n2 = wpool.tile([128, 16, 8], F32, tag="n2")
                nc.scalar.activation(t1[:], VX, AF.Square)
                nc.scalar.activation(t2[:], VY, AF.Square)
                nc.vector.tensor_tensor(n2[:], t1[:], t2[:], op=AL.add)
                nc.scalar.activation(n2[:], n2[:], AF.Sqrt)
                nc.vector.tensor_scalar_max(n2[:], n2[:], 1e-12)
                inv = wpool.tile([128, 16, 8], F32, tag="inv")
                nc.vector.reciprocal(inv[:], n2[:])

                rx = wpool.tile([128, 16, 8], F32, tag="rx")
                ry = wpool.tile([128, 16, 8], F32, tag="ry")
                nc.vector.tensor_tensor(t1[:], VX, wxs3, op=AL.mult)
                nc.vector.tensor_tensor(t2[:], VY, wys3, op=AL.mult)
                nc.vector.tensor_tensor(rx[:], t1[:], t2[:], op=AL.subtract)
                nc.vector.tensor_tensor(t1[:], VY, wxs3, op=AL.mult)
                nc.vector.tensor_tensor(t2[:], VX, wys3, op=AL.mult)
                nc.vector.tensor_tensor(ry[:], t1[:], t2[:], op=AL.add)
                nc.vector.tensor_tensor(rx[:], rx[:], inv[:], op=AL.mult)
                nc.vector.tensor_tensor(ry[:], ry[:], inv[:], op=AL.mult)
                nc.vector.tensor_scalar(rx[:], rx[:], 256.0, 256.0,
                                        op0=AL.mult, op1=AL.add)
                nc.vector.tensor_scalar(rx[:], rx[:], 0.0, 511.0,
                                        op0=AL.max, op1=AL.min)
                nc.vector.tensor_scalar(ry[:], ry[:], 256.0, 256.0,
                                        op0=AL.mult, op1=AL.add)
                nc.vector.tensor_scalar(ry[:], ry[:], 0.0, 511.0,
                                        op0=AL.max, op1=AL.min)
                ixi = wpool.tile([128, 16, 8], I32, tag="ixi")
                iyi = wpool.tile([128, 16, 8], I32, tag="iyi")
                flr = wpool.tile([128, 16, 8], F32, tag="flr")
                fmk = wpool.tile([128, 16, 8], F32, tag="fmk")

                def exact_floor(dst_i, src_f):
                    # dst = floor(src) for src >= 0, independent of cvt rounding
                    nc.vector.tensor_copy(dst_i[:], src_f)
                    nc.vector.tensor_copy(flr[:], dst_i[:])
                    nc.vector.tensor_tensor(fmk[:], flr[:], src_f, op=AL.is_gt)
                    nc.vector.tensor_tensor(flr[:], flr[:], fmk[:], op=AL.subtract)
                    nc.vector.tensor_copy(dst_i[:], flr[:])

                exact_floor(ixi, rx[:])
                exact_floor(iyi, ry[:])

                sig = wpool.tile([128, 16, 8], F32, tag="sig")
                nc.scalar.activation(sig[:], ZZ, AF.Sigmoid)
                nc.vector.tensor_scalar(sig[:], sig[:], float(ANG), 0.5,
                                        op0=AL.mult, op1=AL.add)
                nc.vector.tensor_scalar_min(sig[:], sig[:], 7.9)
                zif = wpool.tile([128, 16, 8], F32, tag="zif")
                zii = wpool.tile([128, 16, 8], I32, tag="zii")
                exact_floor(zii, sig[:])
                nc.vector.tensor_copy(zif[:], zii[:])

                fi3 = fi_all[:, cw:cw + 128].rearrange("p (ch f) -> p ch f", ch=16)
                dcs = dct[:, 0:8].rearrange("p (o f) -> p o f", o=1).to_broadcast([128, 16, 8])
                ixf = wpool.tile([128, 16, 8], F32, tag="ixf")
                iyf = wpool.tile([128, 16, 8], F32, tag="iyf")
                nc.vector.tensor_copy(ixf[:], ixi[:])
                nc.vector.tensor_copy(iyf[:], iyi[:])
                ff = wpool.tile([128, 16, 8], F32, tag="ff")
                nc.vector.scalar_tensor_tensor(ff[:], zif[:], 8.0, dcs,
                                               op0=AL.mult, op1=AL.add)
                nc.vector.scalar_tensor_tensor(ff[:], ff[:], 512.0, iyf[:],
                                               op0=AL.mult, op1=AL.add)
                nc.vector.scalar_tensor_tensor(ff[:], ff[:], 512.0, ixf[:],
                                               op0=AL.mult, op1=AL.add)
                nc.vector.tensor_copy(fi3, ff[:])

                facv = wpool.tile([128, 16, 8], F32, tag="facv")
                nrmv = wpool.tile([128, 16, 8], F32, tag="nrmv")
                msk = wpool.tile([128, 16, 8], F32, tag="msk")
                tmpm = wpool.tile([128, 16, 8], F32, tag="tmpm")
                nc.vector.memset(facv[:], 0.0)
                nc.vector.memset(nrmv[:], 0.0)
                for k in range(ANG):
                    fr = fact[:, k * D:(k + 1) * D].rearrange(
                        "p (o f) -> p o f", o=1).to_broadcast([128, 16, 8])
                    nr = normt[:, k * D:(k + 1) * D].rearrange(
                        "p (o f) -> p o f", o=1).to_broadcast([128, 16, 8])
                    nc.vector.tensor_scalar(msk[:], zif[:], float(k), None,
                                            op0=AL.is_equal)
                    nc.vector.tensor_tensor(tmpm[:], msk[:], fr, op=AL.mult)
                    nc.vector.tensor_tensor(facv[:], facv[:], tmpm[:], op=AL.add)
                    nc.vector.tensor_tensor(tmpm[:], msk[:], nr, op=AL.mult)
                    nc.vector.tensor_tensor(nrmv[:], nrmv[:], tmpm[:], op=AL.add)

                rl = wpool.tile([128, 16, 8], F32, tag="rl")
                wd = wpool.tile([128, 16, 8], F32, tag="wd")
                nc.scalar.activation(rl[:], WT, AF.Relu)
                wn3 = wn_all[:, cw:cw + 128].rearrange("p (ch f) -> p ch f", ch=16)
                nc.vector.tensor_tensor(wn3, rl[:], facv[:], op=AL.mult)
                nc.scalar.activation(wd[:], facv[:], AF.Abs)
                nc.vector.tensor_tensor(wd[:], rl[:], wd[:], op=AL.mult)
                den3 = den_all[:, s * 16:(s + 1) * 16].rearrange("p (c o) -> p c o", o=1)
                nc.vector.tensor_reduce(den3, wd[:],
                                        axis=mybir.AxisListType.X, op=AL.add)
                # fold the norm factor into wn
                nc.vector.tensor_tensor(wn3, wn3, nrmv[:], op=AL.mult)

            # ---- phase 2: gather loop (static indirect APs, dual staging) ----
            UNROLL = 64
            HALF = UNROLL // 2
            fi_s0 = cpool.tile([128, HALF], I32)
            fi_s1 = cpool.tile([128, HALF], I32)
            pv_s0 = cpool.tile([128, HALF], F32)
            pv_s1 = cpool.tile([128, HALF], F32)
            with tc.For_i(0, NCHUNK * D // UNROLL, 1) as iv:
                nc.vector.tensor_copy(
                    fi_s0[:], fi_all[:, bass.ts(iv, UNROLL)][:, 0:HALF])
                nc.vector.tensor_copy(
                    fi_s1[:], fi_all[:, bass.ts(iv, UNROLL)][:, HALF:UNROLL])
                for k in range(HALF):
                    nc.gpsimd.indirect_dma_start(
                        out=pv_s0[:, k:k + 1], out_offset=None,
                        in_=pdf_flat[:],
                        in_offset=bass.IndirectOffsetOnAxis(
                            ap=fi_s0[:, k:k + 1], axis=0))
                    nc.gpsimd.indirect_dma_start(
                        out=pv_s1[:, k:k + 1], out_offset=None,
                        in_=pdf_flat[:],
                        in_offset=bass.IndirectOffsetOnAxis(
                            ap=fi_s1[:, k:k + 1], axis=0))
                nc.vector.tensor_copy(
                    pv_all[:, bass.ts(iv, UNROLL)][:, 0:HALF], pv_s0[:])
                nc.vector.tensor_copy(
                    pv_all[:, bass.ts(iv, UNROLL)][:, HALF:UNROLL], pv_s1[:])

            # ---- phase 3: combine ----
            num = cpool.tile([128, NCHUNK * D], F32)
            nc.vector.tensor_tensor(num[:], wn_all[:], pv_all[:], op=AL.mult)
            nc.vector.tensor_reduce(
                ot[:].rearrange("p (c o) -> p c o", o=1),
                num[:].rearrange("p (c f) -> p c f", f=D),
                axis=mybir.AxisListType.X, op=AL.add)
            nc.vector.tensor_scalar_max(den_all[:], den_all[:], 1e-12)
            nc.vector.reciprocal(den_all[:], den_all[:])
            nc.vector.tensor_tensor(ot[:], ot[:], den_all[:], op=AL.mult)
            nc.sync.dma_start(out_ext[:], ot[:])
    return nc


def prep_inputs(wi, cond, w0, b0, w1, b1, w2, b2, w3, b3, pdf, fac):
    """Host-side sharding + layout. Returns in_maps (list of 8 dicts)."""
    perm = np.concatenate([np.arange(D),                 # weight
                           D + 2 * np.arange(D),         # vx
                           D + 2 * np.arange(D) + 1,     # vy
                           3 * D + np.arange(D)])        # z
    w3r = w3[:, perm].astype(np.float32)
    b3r = b3[perm].astype(np.float32)

    def packw(w, kk):
        t = np.zeros((128, 128), np.float32)
        for g in range(NQ):
            t[32 * g:32 * g + kk, 32 * g:32 * g + 32] = w
        return t

    w0p = packw(w0, CIN); w1p = packw(w1, HID)
    w2p = packw(w2, HID); w3p = packw(w3r, HID)
    b012 = np.zeros((128, 3), np.float32)
    b3p = np.zeros((128, 1), np.float32)
    for g in range(NQ):
        for li, b in enumerate((b0, b1, b2)):
            b012[32 * g:32 * g + 32, li] = b
        b3p[32 * g:32 * g + 32, 0] = b3r
    facb = np.broadcast_to(fac.reshape(1, ANG * D), (128, ANG * D)).copy()
    dcol = np.broadcast_to(np.arange(D, dtype=np.float32).reshape(1, D),
                           (128, D)).copy()
    ident = np.eye(128, dtype=np.float32)
    pdf_flat = pdf.reshape(-1, 1).astype(np.float32)
    pdf2d = pdf.reshape(ANG * D * 128, RES * RES // 128).astype(np.float32)

    in_maps = []
    for c in range(NC_N):
        sl = slice(c * BC, (c + 1) * BC)
        cond_c = cond[sl].reshape(NQ, QS, CIN)
        cond_t = np.ascontiguousarray(
            cond_c.transpose(0, 2, 1).reshape(NQ * CIN, QS))
        # wi per chunk layout: chunk (s,g,cc): sample g*QS + s*512 + cc*128 + p
        wi_c = wi[sl]
        g_, s_, cc_, p_ = np.meshgrid(np.arange(NQ), np.arange(NT), np.arange(4),
                                      np.arange(128), indexing='ij')
        samp = (g_ * QS + s_ * 512 + cc_ * 128 + p_)
        chunk = (s_ * 16 + cc_ * 4 + g_)
        wx = np.zeros((128, NCHUNK, D), np.float32)
        wy = np.zeros((128, NCHUNK, D), np.float32)
        wx[p_.ravel(), chunk.ravel()] = wi_c[samp.ravel(), 0:1]
        wy[p_.ravel(), chunk.ravel()] = wi_c[samp.ravel(), 1:2]
        in_maps.append(dict(
            cond_t=cond_t, wi_x=wx.reshape(128, NCHUNK * D),
            wi_y=wy.reshape(128, NCHUNK * D),
            w0p=w0p, w1p=w1p, w2p=w2p, w3p=w3p, b012=b012, b3p=b3p,
            facb=facb, dcol=dcol, ident=ident,
            pdf_flat=pdf_flat, pdf2d=pdf2d))
    return in_maps


def unshard_output(results):
    out = np.empty(B, np.float32)
    g_, s_, cc_, p_ = np.meshgrid(np.arange(NQ), np.arange(NT), np.arange(4),
                                  np.arange(128), indexing='ij')
    samp = (g_ * QS + s_ * 512 + cc_ * 128 + p_).ravel()
    chunk = (s_ * 16 + cc_ * 4 + g_).ravel()
    for c in range(NC_N):
        o = results[c]["out"]  # [128, NCHUNK]
        out[c * BC + samp] = o[p_.ravel(), chunk]
    return out


_CACHE = {}


def kernel(**inputs):
    if 'nc' not in _CACHE:
        _CACHE['nc'] = build_kernel()
    nc = _CACHE['nc']
    if not nc.is_finalized():
        nc.finalize()
    in_maps = prep_inputs(**{k: np.asarray(v) for k, v in inputs.items()})
    r = run_bass_kernel_spmd(nc, in_maps, list(range(NC_N)),
                             trace=bool(os.environ.get("KTRACE")))
    if r.exec_time_ns:
        print(f"HW exec time: {r.exec_time_ns} ns")
    return unshard_output(r.results)


if __name__ == "__main__":
    pass
